# revision 21
# baseline (speedup 1.0000x reference)
"""MiniGPT forward on 8 Trainium2 NeuronCores.

Sharding: token-split data parallelism with causally-balanced interleaved
query-tile ownership. Core c owns token tiles QSETS[c%2] of sequence
p = c//2 (QSETS = {0,3,4,7} / {1,2,5,6} — both sets see 18 causal key
blocks, so attention work is balanced). Per layer the cores in a pair
AllGather the LN1 output a_in (two half-gathers so K/V GEMMs start as
soon as the first token half lands); each core then computes K and V for
the full sequence locally. V is produced directly in [token, feature]
layout (a_in as the stationary operand), so attention needs no on-chip
transposes. Causal skip: key tile j only processed for query columns
>= S[j] (union of both cores' needs; per-core mask data zeroes the
remainder).

On-chip layout is feature-major: activations live as [128, n_tiles,
TLOC] SBUF tiles, weights stream in pre-transposed (host-side) as
[128, k_tile, out_features] bf16. Logits are written bf16 and widened
on the host.
"""

import numpy as np
import ml_dtypes

P = 128

FULL_CFG = dict(B=4, T=1024, D=1024, H=16, HD=64, L=4, FF=4096, V=32000, NC=8,
                VCH=500, EPS=1e-5)

QSETS = ([0, 3, 4, 7], [1, 2, 5, 6])   # global q-tile ownership by s = c%2


def _derived(cfg):
    d = dict(cfg)
    d["TLOC"] = cfg["B"] * cfg["T"] // cfg["NC"]     # tokens per core
    d["DT"] = cfg["D"] // P                          # d-model tiles
    d["FT"] = cfg["FF"] // P                         # ffn hidden tiles
    d["KTT"] = cfg["T"] // P                         # key tiles (full seq)
    d["MT"] = d["TLOC"] // P                         # token tiles per core
    d["NVC"] = cfg["V"] // cfg["VCH"]                # head vocab chunks
    d["HPT"] = P // cfg["HD"]                        # heads per 128-tile
    assert d["TLOC"] == 512 and d["KTT"] == 8 and d["MT"] == 4
    assert cfg["NC"] == 2 * cfg["B"]
    return d


# ktile j is needed for global q-tiles >= j; with interleaved ownership the
# union over both cores of "first local q-tile >= j" gives these suffix
# starts (in units of local columns):
SUFX = [0, 0, 128, 128, 256, 256, 384, 384]
WJ = [512 - s for s in SUFX]          # processed query cols per ktile
OFF = [sum(WJ[:j]) for j in range(len(WJ))]   # packed col offsets
PTW = sum(WJ)                          # 2560


def build_nc(cfg):
    import concourse.bass as bass
    import concourse.mybir as mybir
    import concourse.tile as tile
    from concourse import bacc

    f32 = mybir.dt.float32
    bf16 = mybir.dt.bfloat16
    AL = mybir.AluOpType
    AF = mybir.ActivationFunctionType

    c = _derived(cfg)
    B, T, D, H, HD, L, FF, V, NC = (cfg[k] for k in
                                    ("B", "T", "D", "H", "HD", "L", "FF", "V", "NC"))
    TLOC, DT, FT, KTT, MT, NVC, HPT, VCH = (
        c[k] for k in ("TLOC", "DT", "FT", "KTT", "MT", "NVC", "HPT", "VCH"))
    EPS = cfg["EPS"]
    QSCALE = 1.0 / np.sqrt(HD)
    groups = [[2 * p, 2 * p + 1] for p in range(NC // 2)]

    # global q-tile -> (owning rank within pair, local index on that rank)
    rk = {}
    for s in range(2):
        for i, g in enumerate(QSETS[s]):
            rk[g] = (s, i)

    nc = bacc.Bacc("TRN2", target_bir_lowering=False, debug=False, num_devices=NC)

    def din(name, shape, dt=f32):
        return nc.dram_tensor(name, list(shape), dt, kind="ExternalInput")

    h0T = din("h0T", [P, DT, TLOC])
    wqkvT = din("wqkvT", [L, P, DT, 3 * D], bf16)
    wprojT = din("wprojT", [L, P, DT, D], bf16)
    wfc1T = din("wfc1T", [L, P, DT, FF], bf16)
    wfc2T = din("wfc2T", [L, P, FT, D], bf16)
    qkv_b = din("qkv_b", [L, P, 3 * DT])
    vb_row = din("vb_row", [L, 1, D], bf16)
    proj_b = din("proj_b", [L, P, DT])
    fc1_b = din("fc1_b", [L, P, FT])
    fc2_b = din("fc2_b", [L, P, DT])
    ln1_g = din("ln1_g", [L, P, DT])
    ln1_b = din("ln1_b", [L, P, DT])
    ln2_g = din("ln2_g", [L, P, DT])
    ln2_b = din("ln2_b", [L, P, DT])
    lnf_g = din("lnf_g", [P, DT])
    lnf_b = din("lnf_b", [P, DT])
    masks = din("masks", [P, KTT, P], bf16)
    embT = din("embT", [P, DT, V], bf16)
    logits = nc.dram_tensor("logits", [TLOC, V], bf16, kind="ExternalOutput")

    with tile.TileContext(nc) as tc:
        import contextlib
        ctx = contextlib.ExitStack()
        with ctx:
            persist = ctx.enter_context(tc.tile_pool(name="persist", bufs=1))
            ppool = ctx.enter_context(tc.tile_pool(name="ppool", bufs=4, space="PSUM"))
            pp2 = ctx.enter_context(tc.tile_pool(name="pp2", bufs=2, space="PSUM"))
            wpool = ctx.enter_context(tc.tile_pool(name="wpool", bufs=2))
            bigpool = ctx.enter_context(tc.tile_pool(name="bigpool", bufs=1))
            actpool = ctx.enter_context(tc.tile_pool(name="actpool", bufs=2))
            qpool = ctx.enter_context(tc.tile_pool(name="qpool", bufs=1))
            kpool = ctx.enter_context(tc.tile_pool(name="kpool", bufs=1))
            vpool = ctx.enter_context(tc.tile_pool(name="vpool", bufs=1))
            apool = ctx.enter_context(tc.tile_pool(name="apool", bufs=1))
            p_pool = ctx.enter_context(tc.tile_pool(name="p_pool", bufs=2))
            rowpool = ctx.enter_context(tc.tile_pool(name="rowpool", bufs=2))
            tmppool = ctx.enter_context(tc.tile_pool(name="tmppool", bufs=2))
            parpool = ctx.enter_context(tc.tile_pool(name="parpool", bufs=2))
            embpool = ctx.enter_context(tc.tile_pool(name="embpool", bufs=2))
            outpool = ctx.enter_context(tc.tile_pool(name="outpool", bufs=2))
            tmp1 = ctx.enter_context(tc.tile_pool(name="tmp1", bufs=1))
            drampool = ctx.enter_context(tc.tile_pool(name="drampool", bufs=2,
                                                      space="DRAM"))

            zero_col = persist.tile([P, 1], f32)
            nc.vector.memset(zero_col[:], 0.0)
            nc.const_aps.aps[(f32, 0.0)] = zero_col[:]
            eps_col = persist.tile([P, 1], f32)
            nc.vector.memset(eps_col[:], EPS)
            nc.const_aps.aps[(f32, EPS)] = eps_col[:]

            ones_f = persist.tile([P, P], f32)
            nc.vector.memset(ones_f[:], 1.0)
            invD_col = persist.tile([P, 1], f32)
            nc.vector.memset(invD_col[:], 1.0 / D)
            ones_b = persist.tile([P, P], bf16)
            nc.vector.memset(ones_b[:], 1.0)

            masks_sb = persist.tile([P, KTT, P], bf16)
            nc.sync.dma_start(masks_sb[:], masks.ap())

            h_sb = persist.tile([P, DT, TLOC], f32)
            nc.sync.dma_start(h_sb[:], h0T.ap())

            attout_sb = persist.tile([P, DT, TLOC], bf16)

            lnf_g_sb = persist.tile([P, DT], f32)
            nc.sync.dma_start(lnf_g_sb[:], lnf_g.ap())
            lnf_b_sb = persist.tile([P, DT], f32)
            nc.sync.dma_start(lnf_b_sb[:], lnf_b.ap())

            # K (feature-major) and V^T (token-major, 65-wide per head with a
            # trailing ones column for the softmax denominator)
            k_sb = persist.tile([P, DT, T], bf16)
            v_sb = persist.tile([P, KTT, H, HD + 1], bf16)
            nc.vector.memset(v_sb[:, :, :, HD:HD + 1], 1.0)

            def act_recip(out, in_):
                ins_l = [nc.scalar.lower_ap(in_)]
                for val in (0.0, 1.0, 0.0):  # bias, scale, alpha
                    ins_l.append(mybir.ImmediateValue(dtype=f32, value=val))
                nc.scalar.add_instruction(mybir.InstActivation(
                    name=nc.get_next_instruction_name(),
                    func=AF.Reciprocal, ins=ins_l,
                    outs=[nc.scalar.lower_ap(out)]))

            def layernorm(g_sb, b_sb, out_bf16_ap_fn, post_tile_fn=None,
                          ntiles=DT):
                """h_sb [P, DT, TLOC] f32 -> bf16 tiles via out_bf16_ap_fn(t)."""
                sum_ps = ppool.tile([1, TLOC], f32, tag="ps", name="ln_sum")
                sq_ps = ppool.tile([1, TLOC], f32, tag="ps", name="ln_sq")
                for t in range(ntiles):
                    nc.tensor.matmul(sum_ps[:], invD_col[:, 0:1], h_sb[:, t, :],
                                     start=(t == 0), stop=(t == ntiles - 1))
                    hsq = tmppool.tile([P, TLOC], f32, tag="hsq", name="hsq")
                    nc.vector.tensor_tensor(hsq[:], h_sb[:, t, :], h_sb[:, t, :],
                                            AL.mult)
                    nc.tensor.matmul(sq_ps[:], invD_col[:, 0:1], hsq[:],
                                     start=(t == 0), stop=(t == ntiles - 1))
                mean = rowpool.tile([1, TLOC], f32, tag="row", name="mean")
                nc.vector.tensor_copy(mean[:], sum_ps[:])
                m2 = rowpool.tile([1, TLOC], f32, tag="row", name="m2")
                nc.vector.tensor_tensor(m2[:], mean[:], mean[:], AL.mult)
                var = rowpool.tile([1, TLOC], f32, tag="row", name="var")
                nc.vector.tensor_tensor(var[:], sq_ps[:], m2[:],
                                        AL.subtract)
                std = rowpool.tile([1, TLOC], f32, tag="row", name="std")
                nc.scalar.activation(std[:], var[:], AF.Sqrt, bias=EPS)
                mean_bc = ppool.tile([P, TLOC], f32, tag="ps", name="mean_bc")
                nc.tensor.matmul(mean_bc[:], ones_f[0:1, :], mean[:],
                                 start=True, stop=True)
                std_bc = ppool.tile([P, TLOC], f32, tag="ps", name="std_bc")
                nc.tensor.matmul(std_bc[:], ones_f[0:1, :], std[:],
                                 start=True, stop=True)
                rstd_sb = tmppool.tile([P, TLOC], f32, tag="rstd", name="rstd_sb")
                act_recip(rstd_sb[:], std_bc[:])
                for t in range(ntiles):
                    tmp = tmppool.tile([P, TLOC], f32, tag="lntmp", name="lntmp")
                    nc.vector.tensor_sub(tmp[:], h_sb[:, t, :], mean_bc[:])
                    nc.vector.tensor_tensor(tmp[:], tmp[:], rstd_sb[:], AL.mult)
                    nc.vector.tensor_scalar(out_bf16_ap_fn(t), tmp[:],
                                            g_sb[:, t:t + 1], b_sb[:, t:t + 1],
                                            AL.mult, AL.add)
                    if post_tile_fn is not None:
                        post_tile_fn(t)

            def load_par(src, l, width):
                t = parpool.tile([P, width], f32, tag=f"par{width}", name="par")
                nc.sync.dma_start(t[:], src.ap()[l])
                return t

            for l in range(L):
                g1 = load_par(ln1_g, l, DT)
                b1 = load_par(ln1_b, l, DT)
                a_in = actpool.tile([P, DT, TLOC], bf16, tag="a_in", name="a_in1")

                # a_in AllGather in two FEATURE halves, each issued as soon as
                # LN1 has produced its d-tiles; K/V GEMMs consume the halves in
                # k-order so the collective latency hides behind LN1 + Q.
                ag_in = [drampool.tile([DT // 2, P, TLOC], bf16, tag="agin",
                                       name=f"ag_in{hf}") for hf in range(2)]
                ag_out = [drampool.tile([2, DT // 2, P, TLOC], bf16, tag="agout",
                                        name=f"ag_out{hf}") for hf in range(2)]

                def ln1_hook(t):
                    if t == 3 or t == DT - 1:
                        hf = 0 if t == 3 else 1
                        nc.sync.dma_start(
                            ag_in[hf][:].rearrange("t p c -> p t c"),
                            a_in[:, hf * 4:hf * 4 + 4, :])
                        nc.gpsimd.collective_compute(
                            "AllGather", AL.bypass, replica_groups=groups,
                            ins=[ag_in[hf][:].opt()], outs=[ag_out[hf][:].opt()])

                layernorm(g1, b1, lambda t: a_in[:, t, :], ln1_hook)

                qb = load_par(qkv_b, l, 3 * DT)
                vbr = parpool.tile([1, D], bf16, tag="vbr", name="vbr")
                nc.sync.dma_start(vbr[:], vb_row.ap()[l])

                # ---- Q GEMM (local tokens, from a_in)
                q_sb = qpool.tile([P, DT, TLOC], bf16, name="q_sb")
                wq_ap = wqkvT.ap()[l]  # [P, DT, 3D]
                for ch in range(2):
                    wt = wpool.tile([P, DT, 4 * P], bf16, tag="w", name="wq")
                    nc.sync.dma_start(wt[:], wq_ap[:, :, ch * 512:(ch + 1) * 512])
                    for m in range(4):
                        mt = ch * 4 + m
                        ps = ppool.tile([P, TLOC], f32, tag="ps", name="q_ps")
                        for k in range(DT):
                            nc.tensor.matmul(ps[:], wt[:, k, m * P:(m + 1) * P],
                                             a_in[:, k, :],
                                             start=(k == 0), stop=(k == DT - 1))
                        nc.vector.tensor_scalar(q_sb[:, mt, :], ps[:],
                                                qb[:, mt:mt + 1], QSCALE,
                                                AL.add, AL.mult)

                # ---- gathered a_in in global token order, per feature half
                a_all = apool.tile([P, DT, T], bf16, name="a_all")
                for hf in range(2):
                    for g in range(KTT):
                        r, i = rk[g]
                        nc.sync.dma_start(
                            a_all[:, hf * 4:hf * 4 + 4, g * P:(g + 1) * P],
                            ag_out[hf][r, :, :, i * P:(i + 1) * P]
                            .rearrange("t p c -> p t c"))

                # K GEMM (full sequence, both token chunks per weight chunk)
                for ch in range(2):
                    wt = wpool.tile([P, DT, 4 * P], bf16, tag="w", name="wk")
                    nc.sync.dma_start(
                        wt[:], wq_ap[:, :, D + ch * 512:D + (ch + 1) * 512])
                    for tc in range(2):
                        for m in range(4):
                            mt = ch * 4 + m
                            ps = ppool.tile([P, TLOC], f32, tag="ps", name="k_ps")
                            for k in range(DT):
                                nc.tensor.matmul(
                                    ps[:], wt[:, k, m * P:(m + 1) * P],
                                    a_all[:, k, tc * 512:(tc + 1) * 512],
                                    start=(k == 0), stop=(k == DT - 1))
                            nc.vector.tensor_scalar(
                                k_sb[:, mt, tc * 512:(tc + 1) * 512], ps[:],
                                qb[:, DT + mt:DT + mt + 1], 1.0, AL.add, AL.mult)

                # V^T GEMM: a_all tiles stationary, V weights moving ->
                # [token, feature]; ones x vb_row seeds the bias.
                for fh in range(2):
                    wt = wpool.tile([P, DT, 4 * P], bf16, tag="w", name="wv")
                    nc.sync.dma_start(
                        wt[:],
                        wq_ap[:, :, 2 * D + fh * 512:2 * D + (fh + 1) * 512])
                    for g in range(KTT):
                        ps = ppool.tile([P, TLOC], f32, tag="ps", name="v_ps")
                        nc.tensor.matmul(ps[:], ones_b[0:1, :],
                                         vbr[:, fh * 512:(fh + 1) * 512],
                                         start=True, stop=False)
                        for k in range(DT):
                            nc.tensor.matmul(
                                ps[:], a_all[:, k, g * P:(g + 1) * P],
                                wt[:, k, :],
                                start=False, stop=(k == DT - 1))
                        nc.vector.tensor_copy(
                            v_sb[:, g, fh * 8:fh * 8 + 8, 0:HD], ps[:])

                # ---- attention: head pairs, causal-skipped blocks. QK of both
                # heads is issued before either AV, so exp (scalar) pipelines
                # behind the PE instead of stalling it. Only the first 128-col
                # block of each ktile window can carry a non-trivial mask.
                for i in range(H // 2):
                    pts = []
                    for h in (2 * i, 2 * i + 1):
                        hb = (h % HPT) * HD
                        ht = h // HPT
                        p_t = p_pool.tile([P, PTW], bf16, tag="P", name="p_t")
                        for j in range(KTT):
                            S, W, O = SUFX[j], WJ[j], OFF[j]
                            sp = ppool.tile([P, TLOC], f32, tag="ps",
                                            name="s_ps")
                            nc.tensor.matmul(
                                sp[:, 0:W],
                                k_sb[hb:hb + HD, ht, j * P:(j + 1) * P],
                                q_sb[hb:hb + HD, ht, S:], start=True, stop=True)
                            nc.scalar.activation(p_t[:, O:O + W], sp[:, 0:W],
                                                 AF.Exp)
                            nc.vector.tensor_tensor(
                                p_t[:, O:O + P], p_t[:, O:O + P],
                                masks_sb[:, j, :], AL.mult)
                        pts.append((h, hb, ht, p_t))
                    aps = []
                    for (h, hb, ht, p_t) in pts:
                        ap = ppool.tile([P, TLOC], f32, tag="ps", name="av_ps")
                        for j in range(KTT):
                            S, W, O = SUFX[j], WJ[j], OFF[j]
                            nc.tensor.matmul(ap[0:HD + 1, S:], v_sb[:, j, h, :],
                                             p_t[:, O:O + W],
                                             start=(j == 0),
                                             stop=(j == KTT - 1))
                        aps.append((hb, ht, ap))
                    den2 = rowpool.tile([1, 2 * TLOC], f32, tag="row2",
                                        name="den2")
                    for q, (hb, ht, ap) in enumerate(aps):
                        nc.vector.tensor_copy(den2[:, q * TLOC:(q + 1) * TLOC],
                                              ap[HD:HD + 1, :])
                    rec2 = rowpool.tile([1, 2 * TLOC], f32, tag="row2",
                                        name="rec2")
                    act_recip(rec2[:], den2[:])
                    for q, (hb, ht, ap) in enumerate(aps):
                        db = ppool.tile([P, TLOC], f32, tag="ps", name="db_ps")
                        nc.tensor.matmul(db[0:HD, :], ones_f[0:1, 0:HD],
                                         rec2[:, q * TLOC:(q + 1) * TLOC],
                                         start=True, stop=True)
                        db_sb = tmppool.tile([HD, TLOC], f32, tag="rb_sb",
                                             name="db_sb")
                        nc.vector.tensor_copy(db_sb[:], db[0:HD, :])
                        nc.vector.tensor_tensor(attout_sb[hb:hb + HD, ht, :],
                                                ap[0:HD, :], db_sb[:], AL.mult)

                # ---- proj GEMM + residual
                pb = load_par(proj_b, l, DT)
                for ch in range(DT // 4):
                    wt = wpool.tile([P, DT, 4 * P], bf16, tag="w", name="wproj")
                    nc.sync.dma_start(
                        wt[:], wprojT.ap()[l][:, :, ch * 512:(ch + 1) * 512])
                    for m in range(4):
                        mt = ch * 4 + m
                        ps = ppool.tile([P, TLOC], f32, tag="ps", name="proj_ps")
                        for k in range(DT):
                            nc.tensor.matmul(ps[:], wt[:, k, m * P:(m + 1) * P],
                                             attout_sb[:, k, :],
                                             start=(k == 0), stop=(k == DT - 1))
                        nc.vector.scalar_tensor_tensor(h_sb[:, mt, :], ps[:],
                                                       pb[:, mt:mt + 1],
                                                       h_sb[:, mt, :],
                                                       AL.add, AL.add)

                # ---- mlp
                g2 = load_par(ln2_g, l, DT)
                b2 = load_par(ln2_b, l, DT)
                a2 = actpool.tile([P, DT, TLOC], bf16, tag="a_in", name="a_in2")
                layernorm(g2, b2, lambda t: a2[:, t, :])

                f1b = load_par(fc1_b, l, FT)
                mact = bigpool.tile([P, FT, TLOC], bf16, tag="big", name="mact")
                w1_ap = wfc1T.ap()[l]  # [P, DT, FF]
                for ch in range(FT // 4):
                    wt = wpool.tile([P, DT, 4 * P], bf16, tag="w", name="wfc1")
                    nc.sync.dma_start(wt[:],
                                      w1_ap[:, :, ch * 512:(ch + 1) * 512])
                    for m in range(4):
                        mt = ch * 4 + m
                        ps = ppool.tile([P, TLOC], f32, tag="ps", name="fc1_ps")
                        for k in range(DT):
                            nc.tensor.matmul(ps[:], wt[:, k, m * P:(m + 1) * P],
                                             a2[:, k, :],
                                             start=(k == 0), stop=(k == DT - 1))
                        nc.scalar.activation(mact[:, mt, :], ps[:], AF.Gelu,
                                             bias=f1b[:, mt:mt + 1])

                f2b = load_par(fc2_b, l, DT)
                w2_ap = wfc2T.ap()[l]  # [P, FT, D]
                CH2 = 4
                MG = 4  # m-tiles per pass
                for mg in range(0, DT, MG):
                    ps_t = [pp2.tile([P, 2, TLOC], f32, tag="ps2",
                                     name=f"fc2_ps{mg}_{m2}")
                            for m2 in range((MG + 1) // 2)]
                    for ch in range(FT // CH2):
                        wt = wpool.tile([P, CH2, MG * P], bf16, tag="w",
                                        name="wfc2")
                        nc.sync.dma_start(
                            wt[:], w2_ap[:, ch * CH2:(ch + 1) * CH2,
                                         mg * P:(mg + MG) * P])
                        for m in range(MG):
                            for k in range(CH2):
                                kt = ch * CH2 + k
                                nc.tensor.matmul(
                                    ps_t[m // 2][:, m % 2, :],
                                    wt[:, k, m * P:(m + 1) * P],
                                    mact[:, kt, :],
                                    start=(kt == 0), stop=(kt == FT - 1))
                    for m in range(MG):
                        nc.vector.scalar_tensor_tensor(
                            h_sb[:, mg + m, :], ps_t[m // 2][:, m % 2, :],
                            f2b[:, mg + m:mg + m + 1], h_sb[:, mg + m, :],
                            AL.add, AL.add)

            # ---- final layernorm + tied head
            af = actpool.tile([P, DT, TLOC], bf16, tag="a_in", name="a_f")
            layernorm(lnf_g_sb, lnf_b_sb, lambda t: af[:, t, :])

            # psum drains alternate scalar / vector so neither engine gates
            # the 8-bank turnover; logits leave the core as bf16.
            for vc in range(NVC):
                ec = embpool.tile([P, DT, VCH], bf16, tag="emb", name="ec")
                nc.sync.dma_start(ec[:], embT.ap()[:, :, vc * VCH:(vc + 1) * VCH])
                for m in range(MT):
                    ps = ppool.tile([P, VCH], f32, tag="ps", name="head_ps")
                    for k in range(DT):
                        nc.tensor.matmul(ps[:], af[:, k, m * P:(m + 1) * P],
                                         ec[:, k, :],
                                         start=(k == 0), stop=(k == DT - 1))
                    ls = outpool.tile([P, VCH], bf16, tag="lout", name="ls")
                    if m % 2 == 0:
                        nc.scalar.copy(ls[:], ps[:])
                    else:
                        nc.vector.tensor_copy(ls[:], ps[:])
                    nc.sync.dma_start(
                        logits.ap()[m * P:(m + 1) * P, vc * VCH:(vc + 1) * VCH],
                        ls[:])

    nc.compile()
    return nc


# ---------------------------------------------------------------------------
# host side
# ---------------------------------------------------------------------------

_CACHE = {}


def get_nc(cfg_key_and_cfg=None):
    cfg = FULL_CFG if cfg_key_and_cfg is None else cfg_key_and_cfg
    key = tuple(sorted(cfg.items()))
    if key not in _CACHE:
        _CACHE[key] = build_nc(cfg)
    return _CACHE[key]


def host_prep(inputs, cfg):
    """Build the per-core in_maps from full (unsharded) numpy inputs."""
    bf = ml_dtypes.bfloat16
    c = _derived(cfg)
    B, T, D, L, FF, V, NC = (cfg[k] for k in ("B", "T", "D", "L", "FF", "V", "NC"))
    TLOC, DT, FT, KTT = (c[k] for k in ("TLOC", "DT", "FT", "KTT"))

    f = {k: np.asarray(v) for k, v in inputs.items()}
    x = f["x"].astype(np.int64)
    tok = f["tok_emb"].astype(np.float32)
    pos = f["pos_emb"].astype(np.float32)

    def wT_r(w, kdim, fdim):
        # [L, fdim, kdim] -> [L, 128, kdim/128, fdim] bf16
        wt = w.astype(np.float32).transpose(0, 2, 1)          # [L, kdim, fdim]
        wt = wt.reshape(L, kdim // P, P, fdim).transpose(0, 2, 1, 3)
        return np.ascontiguousarray(wt).astype(bf)

    def par_r(b, n):
        # [L, n*128] -> [L, 128, n]
        return np.ascontiguousarray(
            b.astype(np.float32).reshape(L, n, P).transpose(0, 2, 1))

    shared = {
        "wqkvT": wT_r(f["qkv_w"], D, 3 * D),
        "wprojT": wT_r(f["proj_w"], D, D),
        "wfc1T": wT_r(f["fc1_w"], D, FF),
        "wfc2T": wT_r(f["fc2_w"], FF, D),
        "qkv_b": par_r(f["qkv_b"], 3 * DT),
        "vb_row": np.ascontiguousarray(
            f["qkv_b"].astype(np.float32)[:, 2 * D:3 * D]
            .reshape(L, 1, D)).astype(bf),
        "proj_b": par_r(f["proj_b"], DT),
        "fc1_b": par_r(f["fc1_b"], FT),
        "fc2_b": par_r(f["fc2_b"], DT),
        "ln1_g": par_r(f["ln1_g"], DT),
        "ln1_b": par_r(f["ln1_b"], DT),
        "ln2_g": par_r(f["ln2_g"], DT),
        "ln2_b": par_r(f["ln2_b"], DT),
        "lnf_g": np.ascontiguousarray(
            f["lnf_g"].astype(np.float32).reshape(DT, P).T),
        "lnf_b": np.ascontiguousarray(
            f["lnf_b"].astype(np.float32).reshape(DT, P).T),
        "embT": np.ascontiguousarray(
            tok.T.reshape(DT, P, V).transpose(1, 0, 2)).astype(bf),
    }

    in_maps = []
    for core in range(NC):
        p, s = core // 2, core % 2
        qt = QSETS[s]
        rows = np.concatenate([np.arange(g * P, (g + 1) * P) for g in qt])
        h0 = tok[x[p]] + pos[:T]                              # [T, D]
        h0 = h0[rows]                                         # [TLOC, D]
        h0T = np.ascontiguousarray(
            h0.T.reshape(DT, P, TLOC).transpose(1, 0, 2)).astype(np.float32)
        # causal mask for the first 128-col block of each ktile window (the
        # only block that can be non-trivial with union suffix starts)
        m = np.empty((P, KTT, P), np.float32)
        for j in range(KTT):
            keys = j * P + np.arange(P)                       # global key idx
            qcols = rows[SUFX[j]:SUFX[j] + P]                 # global q idx
            m[:, j, :] = (keys[:, None] <= qcols[None, :])
        in_maps.append(dict(shared, h0T=h0T, masks=m.astype(bf)))
    return in_maps


def assemble(results, cfg):
    c = _derived(cfg)
    B, T, V = cfg["B"], cfg["T"], cfg["V"]
    TLOC = c["TLOC"]
    out = np.empty((B, T, V), np.float32)
    for core, r in enumerate(results):
        p, s = core // 2, core % 2
        lg = np.asarray(r["logits"]).astype(np.float32)
        for i, g in enumerate(QSETS[s]):
            out[p, g * P:(g + 1) * P, :] = lg[i * P:(i + 1) * P]
    return out


def run(inputs, cfg=None, **run_kwargs):
    from concourse.bass_utils import run_bass_kernel_spmd
    cfg = cfg or FULL_CFG
    nc = get_nc(cfg)
    in_maps = host_prep(inputs, cfg)
    res = run_bass_kernel_spmd(nc, in_maps, core_ids=list(range(cfg["NC"])),
                               **run_kwargs)
    return assemble(res.results, cfg), res


def kernel(**inputs) -> np.ndarray:
    out, _ = run(inputs, FULL_CFG)
    return out
